# revision 7
# baseline (speedup 1.0000x reference)
"""Trainium2 Bass kernel for BodyStructureLoss.

Computes: mean over (B, J) of where(||kps[b,j,:]|| > 1.0, ||kps[b,j,:]||, 0)
for kps of shape [524288, 17, 3] float32.

Strategy (data-parallel over 8 NeuronCores):
  - Each core gets B/8 = 65536 batch rows = 3,342,336 contiguous floats,
    viewed as [128 partitions, 26112] (each partition row holds 8704
    complete (x,y,z) triplets).
  - Per [128, F] tile:  ACT squares all elements; DVE sums the 3 squared
    components with two strided adds; ACT takes sqrt; DVE builds the
    (s > 1) mask and does a fused multiply+reduce (tensor_tensor_reduce)
    into a per-tile [128, 1] accumulator column.
  - Final on-device reduce over tile columns -> [128, 1] per core; host
    sums 8 x 128 partials and divides by B*J.
"""

import numpy as np

import concourse.bass as bass
import concourse.bacc as bacc
import concourse.mybir as mybir
from concourse.bass_utils import run_bass_kernel_spmd
from concourse.tile import TileContext

B, J, D = 524288, 17, 3
HALF_BODY = 1.0  # threshold/2 with threshold=2.0
N_CORES = 8
B_SHARD = B // N_CORES  # 65536
P = 128
FLOATS_PER_CORE = B_SHARD * J * D  # 3342336
COLS = FLOATS_PER_CORE // P  # 26112 (divisible by 3: 26112 = 3*8704)
N_TILES = 8
F = COLS // N_TILES  # 3264 = 3*1088
M = F // 3  # 1088 triplets per partition per tile

_DT = mybir.dt.float32


def build_nc(P=P, COLS=COLS, n_tiles=N_TILES):
    assert COLS % n_tiles == 0
    F = COLS // n_tiles
    assert F % 3 == 0
    M = F // 3

    nc = bacc.Bacc(
        "TRN2", target_bir_lowering=False, debug=False, num_devices=N_CORES
    )
    x = nc.dram_tensor("x", [P, COLS], _DT, kind="ExternalInput")
    out = nc.dram_tensor("out", [P, 1], _DT, kind="ExternalOutput")

    with TileContext(nc) as tc:
        with (
            tc.tile_pool(name="xin", bufs=3) as xpool,
            tc.tile_pool(name="sqp", bufs=2) as sqpool,
            tc.tile_pool(name="small", bufs=3) as spool,
            tc.tile_pool(name="accp", bufs=1) as accpool,
        ):
            # two accumulator columns per tile: sum(max(d,1)) and count(s>1)
            accs = accpool.tile([P, 2 * n_tiles], _DT)
            total = accpool.tile([P, 1], _DT)

            for i in range(n_tiles):
                xt = xpool.tile([P, F], _DT, tag="xt")
                nc.sync.dma_start(out=xt, in_=x[:, i * F : (i + 1) * F])

                sq = sqpool.tile([P, F], _DT, tag="sq")
                nc.scalar.activation(
                    out=sq, in_=xt, func=mybir.ActivationFunctionType.Square
                )
                sq3 = sq.rearrange("p (m t) -> p m t", t=3)

                s = spool.tile([P, M], _DT, tag="s")
                nc.vector.tensor_tensor(
                    out=s, in0=sq3[:, :, 0], in1=sq3[:, :, 1], op=mybir.AluOpType.add
                )
                nc.vector.tensor_tensor(
                    out=s, in0=s, in1=sq3[:, :, 2], op=mybir.AluOpType.add
                )

                d = spool.tile([P, M], _DT, tag="d")
                nc.scalar.activation(
                    out=d, in_=s, func=mybir.ActivationFunctionType.Sqrt
                )

                # accum = sum_p max(d, 1)   (= sum relu(d-1) + M per partition)
                dmax = spool.tile([P, M], _DT, tag="dmax")
                nc.vector.tensor_scalar(
                    out=dmax,
                    in0=d,
                    scalar1=float(HALF_BODY),
                    scalar2=None,
                    op0=mybir.AluOpType.max,
                    op1=mybir.AluOpType.add,
                    accum_out=accs[:, 2 * i : 2 * i + 1],
                )
                # accum = count(s > 1)
                cnt = spool.tile([P, M], _DT, tag="cnt")
                nc.vector.tensor_scalar(
                    out=cnt,
                    in0=s,
                    scalar1=float(HALF_BODY * HALF_BODY),
                    scalar2=None,
                    op0=mybir.AluOpType.is_gt,
                    op1=mybir.AluOpType.add,
                    accum_out=accs[:, 2 * i + 1 : 2 * i + 2],
                )

            nc.vector.tensor_reduce(
                out=total, in_=accs, axis=mybir.AxisListType.X, op=mybir.AluOpType.add
            )
            nc.sync.dma_start(out=out[:, :], in_=total)

    nc.compile()
    return nc


_nc_cache = None
last_results = None


def kernel(kps_world_pred: np.ndarray) -> np.ndarray:
    global _nc_cache, last_results
    x = np.ascontiguousarray(kps_world_pred, dtype=np.float32)
    assert x.shape == (B, J, D)

    shards = x.reshape(N_CORES, P, COLS)
    in_maps = [{"x": shards[c]} for c in range(N_CORES)]

    if _nc_cache is None:
        _nc_cache = build_nc()

    res = run_bass_kernel_spmd(_nc_cache, in_maps, list(range(N_CORES)))
    last_results = res

    # per-partition device partials hold sum(max(d,1)) + count(s>1)
    #   = masked_sum + n_triplets, so subtract the global triplet count.
    total = np.float64(0.0)
    for c in range(N_CORES):
        total += res.results[c]["out"].astype(np.float64).sum()
    total -= np.float64(B * J)
    return np.asarray(total / (B * J), dtype=np.float32)
